# revision 31
# baseline (speedup 1.0000x reference)
"""Causal self-attention (B=2, T=2048, D=1024, H=16) on 8 trn2 NeuronCores.

Sharding: core = b*4 + g  (b = batch 0/1, g = head-group of 4 heads).
Each core computes its 4 heads' attention for its batch plus the partial
output projection (Wproj rows for those heads); host sums the 4 partials
per batch (the tensor-parallel all-reduce).

All matmul operands are bf16 (psum accumulation stays f32): halves DMA
bytes, 1 cyc/row LDWEIGHTS, and much lower PE power than f32r (which
triggered hardware activity-throttling to ~50-70% util).

The attention inner loop is SOFTWARE-PIPELINED: the PE issue order per
key-block is  sc(jb) -> av(jb-1)  so the PE streams the next block's
scores while the Activation engine computes exp of the current block;
the old order (sc(jb) -> av(jb), in-order PE queue) stalled the PE on
the full sc->exp->mask->av chain (~2.2us/block).  Normalization of unit
(c,p) and the output projection of chunk c-1 are likewise deferred into
the early blocks of the next unit so their PE ops never stall.

Stage 1: per-kt DMA arrivals drive a kt-OUTER k-projection pass (8 psum
groups), so the PE works as soon as the first 128-row slice of xT/w
lands; the v pass runs from resident SBUF; q chunks are projected
inside the attention loop right before use.

Per-core layouts:
  xT      [1024, 2048]   x[b] transposed, bf16     -> sbuf xt  [128, 8*2048]
  wqkv    [1024, 768]    [q 4h | k 4h | v' 4h]     -> sbuf w   [128, 8*768]
          (v' columns pair-swapped: [h1 h0 h3 h2] so the psum v block is
           [odd | even] per pair, matching the v-tile copy layout)
  wproj   [128, 2048]    pair-major Wproj rows     -> sbuf     [128, 2*1024]
  out     [2048, 1024]   partial projection output (f32)

qT/kT pair tiles [128, 2048]: head-even rows 0:64, head-odd rows 64:128
(row-tiled K=64 matmuls use SBUF partitions 0-63 / 64-127).

v tiles: ONE tile vv [128, 16, 2, 194]; per (key-block jb, pair) block:
  [one | zeros*63 | v_o (64) | v_e (64) | one | pad]
  odd  AV lhsT = block[0:128]   -> sumexp row 0,  out rows 64:128
  even AV lhsT = block[128:193] -> out rows 0:64, sumexp row 64
  per-tt psum->sbuf fill is ONE DVE copy into block[64:192] (both pairs).

Softmax normalization: sumexp rows copied psum->sbuf as f32r (DVE),
broadcast to 128 partitions with K=1 ones-matmuls (f32r), fast DVE
reciprocal on the wide broadcast, one multiply per head half.
"""

import os
import numpy as np
import ml_dtypes

DEBUG = os.environ.get("ATTN_DEBUG", "0") == "1"

B, T, D, H, DH = 2, 2048, 1024, 16, 64
HPG = 4          # heads per group (per core)
NKT = D // 128   # 8 contraction tiles over D
NTT = T // 128   # 16 tiles over T (also key blocks)
NC_ = 4          # 4 i-chunks of 512 queries
VBS = 194        # v-tile per-(jb,pair) block: 64 static + 128 v + one + pad
SCALE = 1.0 / np.sqrt(DH)

_PROG = None


def _build_program():
    from contextlib import ExitStack
    from concourse import bacc, mybir, tile

    f32 = mybir.dt.float32
    f32r = mybir.dt.float32r
    bf16 = mybir.dt.bfloat16
    Exp = mybir.ActivationFunctionType.Exp

    nc = bacc.Bacc(
        "TRN2", target_bir_lowering=False, debug=False, enable_asserts=False,
        num_devices=8,
    )
    xT_d = nc.dram_tensor("xT", [D, T], bf16, kind="ExternalInput").ap()
    wqkv_d = nc.dram_tensor("wqkv", [D, 3 * HPG * DH], bf16, kind="ExternalInput").ap()
    wproj_d = nc.dram_tensor("wproj", [128, 2 * D], bf16, kind="ExternalInput").ap()
    mask_d = nc.dram_tensor("mask", [128, 256], bf16, kind="ExternalInput").ap()
    vinit_d = nc.dram_tensor("vinit", [128, NTT * 2 * VBS], bf16,
                             kind="ExternalInput").ap()
    ones_d = nc.dram_tensor("ones", [128, 128], f32r, kind="ExternalInput").ap()
    out_d = nc.dram_tensor("out", [T, D], f32, kind="ExternalOutput").ap()
    if DEBUG:
        dbg = {
            "dbg_q": nc.dram_tensor("dbg_q", [128, 512], bf16,
                                    kind="ExternalOutput").ap(),
            "dbg_k": nc.dram_tensor("dbg_k", [128, 512], bf16,
                                    kind="ExternalOutput").ap(),
            "dbg_ex0": nc.dram_tensor("dbg_ex0", [128, 1024], bf16,
                                      kind="ExternalOutput").ap(),
            "dbg_vv": nc.dram_tensor("dbg_vv", [128, 2 * VBS], bf16,
                                     kind="ExternalOutput").ap(),
            "dbg_av": nc.dram_tensor("dbg_av", [128, 1024], f32,
                                     kind="ExternalOutput").ap(),
            "dbg_dn": nc.dram_tensor("dbg_dn", [65, 512], f32,
                                     kind="ExternalOutput").ap(),
            "dbg_bcs": nc.dram_tensor("dbg_bcs", [128, 1024], f32,
                                      kind="ExternalOutput").ap(),
            "dbg_rc": nc.dram_tensor("dbg_rc", [128, 1024], f32,
                                     kind="ExternalOutput").ap(),
            "dbg_att": nc.dram_tensor("dbg_att", [128, 512], bf16,
                                      kind="ExternalOutput").ap(),
        }

    with tile.TileContext(nc) as tc, ExitStack() as ctx:
        # ---- persistent pools -------------------------------------------
        const_pool = ctx.enter_context(tc.tile_pool(name="const", bufs=1))
        qk_pool = ctx.enter_context(tc.tile_pool(name="qk", bufs=1))
        v_pool = ctx.enter_context(tc.tile_pool(name="v", bufs=1))

        mask_sb = const_pool.tile([128, 256], bf16, tag="mask")
        ones_sb = const_pool.tile([128, 128], f32r, tag="ones")
        wproj_sb = const_pool.tile([128, 2 * D], bf16, tag="wproj")

        # per-chunk q/k tiles: qk_t[(qk, p, c)] = [128, 512]
        qk_t = {}
        for qk in range(2):
            for p in range(2):
                for c in range(NC_):
                    qk_t[qk, p, c] = qk_pool.tile(
                        [128, 512], bf16, tag=f"qk{qk}{p}{c}",
                        name=f"qkt{qk}{p}{c}")
        # all v tiles in one [128, jb, pair, VBS] tile
        vv = v_pool.tile([128, NTT, 2, VBS], bf16, tag="vv", name="vv")

        # ---- stage 1: k + v projection ----------------------------------
        exp_pool = ctx.enter_context(tc.tile_pool(name="exp", bufs=8))
        xt_pool = ctx.enter_context(tc.tile_pool(name="xt", bufs=1))
        wq_pool = ctx.enter_context(tc.tile_pool(name="wq", bufs=1))
        xt_sb = xt_pool.tile([128, NKT * T], bf16, tag="xt")
        w_sb = wq_pool.tile([128, NKT * 768], bf16, tag="w")
        for kt in range(NKT):
            nc.sync.dma_start(
                xt_sb[:, kt * T:(kt + 1) * T],
                xT_d[kt * 128:(kt + 1) * 128, :]
            )
            nc.sync.dma_start(
                w_sb[:, kt * 768:(kt + 1) * 768],
                wqkv_d[kt * 128:(kt + 1) * 128, :],
            )
        nc.sync.dma_start(
            vv.rearrange("p a b c -> p (a b c)"), vinit_d[:])
        nc.sync.dma_start(mask_sb[:], mask_d[:])
        nc.sync.dma_start(ones_sb[:], ones_d[:])
        nc.sync.dma_start(wproj_sb[:], wproj_d[:])

        # ---- pass A: k projection, kt-OUTER so each DMA arrival feeds ---
        # 8 matmuls (one per (c,p) psum group) immediately.
        with tc.tile_pool(name="pk", bufs=1, space="PSUM") as pk_pool:
            pk = {}
            for c in range(NC_):
                for p in range(2):
                    pk[p, c] = pk_pool.tile([128, 512], f32, tag=f"pk{p}{c}",
                                            name=f"pk{p}{c}")
            for kt in range(NKT):
                for c in range(NC_):
                    for p in range(2):
                        nc.tensor.matmul(
                            pk[p, c][:],
                            lhsT=w_sb[:, kt * 768 + 256 + p * 128:
                                      kt * 768 + 256 + p * 128 + 128],
                            rhs=xt_sb[:, kt * T + c * 512:
                                      kt * T + c * 512 + 512],
                            start=(kt == 0), stop=(kt == NKT - 1),
                            skip_group_check=True,
                        )
            for c in range(NC_):
                for p in range(2):
                    nc.scalar.copy(qk_t[1, p, c][:], pk[p, c][:])

        # ---- stage 2+3: pipelined attention -----------------------------
        # (the v and q projections for chunk c are emitted inside the
        # chunk loop, so attention/exp work starts right after pass A and
        # the projection matmuls fill PE slack whenever the Activation
        # engine paces the per-block exp chain)
        psc_pool = ctx.enter_context(
            tc.tile_pool(name="psc", bufs=2, space="PSUM"))
        att_pool = ctx.enter_context(tc.tile_pool(name="att", bufs=1))
        att_t = {}
        for p in range(2):
            for c in range(NC_):
                att_t[p, c] = att_pool.tile([128, 512], bf16, tag=f"att{p}{c}",
                                            name=f"attt{p}{c}")

        with (
            tc.tile_pool(name="dn", bufs=2) as dn_pool,
            tc.tile_pool(name="rc", bufs=2) as rc_pool,
            tc.tile_pool(name="ot", bufs=2) as ot_pool,
            tc.tile_pool(name="pav", bufs=2, space="PSUM") as pav_pool,
        ):
            def emit_norm_a(pend):
                """Normalize stage a: copy the sumexp rows psum->sbuf."""
                av_ = pend[2]
                dn = dn_pool.tile([65, 512], f32r, tag="dn", name="dn")
                nc.vector.tensor_copy(dn[64:65, :], av_[64:65, 0:512])
                nc.vector.tensor_copy(dn[0:1, :], av_[0:1, 512:1024])
                return dn

            def emit_norm_b(pend, dn):
                """Normalize stage b: att = av / sumexp.

                The denominators are broadcast to 128 partitions with K=1
                ones-matmuls, staged psum->sbuf with a plain copy, and
                reciprocal'd from SBUF: the fast DVE reciprocal reads
                garbage from PSUM on real hardware (CoreSim models it
                fine), so it must only ever see SBUF inputs at base
                partition 0.
                """
                c_, p_, av_ = pend
                bc = psc_pool.tile([128, 1024], f32, tag="sc", name="bc")
                nc.tensor.matmul(bc[:, 0:512], lhsT=ones_sb[64:65, :],
                                 rhs=dn[64:65, :], start=True, stop=True)
                nc.tensor.matmul(bc[:, 512:1024], lhsT=ones_sb[0:1, :],
                                 rhs=dn[0:1, :], start=True, stop=True)
                bcs = rc_pool.tile([128, 1024], f32, tag="bcs", name="bcs")
                nc.vector.tensor_copy(bcs[0:64, :], bc[0:64, :])
                rc = rc_pool.tile([128, 1024], f32, tag="rc", name="rc")
                nc.vector.reciprocal_approx_fast(rc[0:64, :], bcs[0:64, :])
                # move the odd-half reciprocals to partitions 64:128 (DMA is
                # the only partition-crossing mover outside the PE)
                nc.sync.dma_start(rc[64:128, 0:512], rc[0:64, 512:1024])
                nc.vector.tensor_mul(
                    att_t[p_, c_][0:64, :], av_[0:64, 0:512], rc[0:64, 0:512])
                nc.vector.tensor_mul(
                    att_t[p_, c_][64:128, :], av_[64:128, 512:1024],
                    rc[64:128, 0:512])
                if DEBUG and (c_, p_) == (0, 0):
                    ava = dn_pool.tile([128, 1024], f32, tag="ava", name="ava")
                    nc.vector.tensor_copy(ava[:], av_[:])
                    nc.sync.dma_start(dbg["dbg_av"], ava[:])
                    nc.sync.dma_start(dbg["dbg_dn"][0:65, :], dn.bitcast(f32))
                    nc.sync.dma_start(dbg["dbg_rc"], rc[:])
                    nc.sync.dma_start(dbg["dbg_att"], att_t[0, 0][:])

            def emit_proj(c_, tt, use_psc):
                """Project output t-tile tt from att chunk c_."""
                pool = psc_pool if use_psc else pav_pool
                pp = pool.tile([128, 1024], f32, tag="sc" if use_psc else "av",
                               name="pp")
                ot = ot_pool.tile([128, D], f32, tag="ot", name="ot")
                for ch in range(2):
                    for p in range(2):
                        nc.tensor.matmul(
                            pp[:, ch * 512:ch * 512 + 512],
                            lhsT=att_t[p, tt // 4][:, (tt % 4) * 128:
                                                   (tt % 4) * 128 + 128],
                            rhs=wproj_sb[:, p * D + ch * 512:
                                         p * D + ch * 512 + 512],
                            start=(p == 0), stop=(p == 1),
                        )
                    nc.vector.tensor_copy(
                        ot[:, ch * 512:ch * 512 + 512],
                        pp[:, ch * 512:ch * 512 + 512])
                    nc.sync.dma_start(
                        out_d[tt * 128:tt * 128 + 128,
                              ch * 512:ch * 512 + 512],
                        ot[:, ch * 512:ch * 512 + 512])

            def emit_v(c_):
                """v projection for chunk c_'s 4 key blocks (one psum tile
                holds all 4 tt groups; single DVE fill copy into vv)."""
                pv = pav_pool.tile([128, 1024], f32, tag="av", name=f"pv{c_}")
                for ttl in range(4):
                    emit_v_group(c_, ttl, pv)
                emit_v_fill(c_, pv)

            def emit_v_group(c_, ttl, pv):
                tt = 4 * c_ + ttl
                for kt in range(NKT):
                    nc.tensor.matmul(
                        pv[:, ttl * 256:ttl * 256 + 256],
                        lhsT=xt_sb[:, kt * T + tt * 128:
                                   kt * T + tt * 128 + 128],
                        rhs=w_sb[:, kt * 768 + 512:kt * 768 + 768],
                        start=(kt == 0), stop=(kt == NKT - 1),
                        skip_group_check=True,
                    )

            def emit_v_fill(c_, pv):
                pv4 = pv.rearrange("p (t pr c) -> p t pr c", t=4, pr=2)
                nc.vector.tensor_copy(
                    vv[:, 4 * c_:4 * c_ + 4, :, 64:192], pv4[:])

            def emit_q_half(c_, p_, qp):
                for kt in range(NKT):
                    nc.tensor.matmul(
                        qp[:, p_ * 512:p_ * 512 + 512],
                        lhsT=w_sb[:, kt * 768 + p_ * 128:
                                  kt * 768 + p_ * 128 + 128],
                        rhs=xt_sb[:, kt * T + c_ * 512:kt * T + c_ * 512 + 512],
                        start=(kt == 0), stop=(kt == NKT - 1),
                        skip_group_check=True,
                    )

            def emit_q(c_):
                qp = psc_pool.tile([128, 1024], f32, tag="sc", name=f"qp{c_}")
                for p_ in range(2):
                    emit_q_half(c_, p_, qp)
                for p_ in range(2):
                    nc.vector.tensor_copy(qk_t[0, p_, c_][:],
                                          qp[:, p_ * 512:p_ * 512 + 512])

            pending = None  # (c, p, av tile) awaiting normalization
            # sprinkle state: v/q work for chunk c+1 emitted between the
            # blocks of unit (c,1) so the PE has work whenever the
            # Activation engine paces the exp chain.
            spr_pv = None
            for c in range(NC_):
                if c == 0:
                    emit_q(0)
                    emit_v(0)

                for p in range(2):
                    av = pav_pool.tile([128, 1024], f32, tag="av", name="av")
                    njb = 4 * c + 4
                    ex_t = [None] * njb
                    off_t = [0] * njb

                    def emit_av(jb):
                        o = off_t[jb]
                        nc.tensor.matmul(
                            av[0:65, o:512],
                            lhsT=vv[:, jb, p, 128:193],
                            rhs=ex_t[jb][:, o:512],
                            start=(jb == 0), stop=(jb == njb - 1),
                            skip_group_check=True,
                        )
                        nc.tensor.matmul(
                            av[:, 512 + o:1024],
                            lhsT=vv[:, jb, p, 0:128],
                            rhs=ex_t[jb][:, 512 + o:1024],
                            start=(jb == 0), stop=(jb == njb - 1),
                            skip_group_check=True,
                        )

                    for jb in range(njb):
                        r = jb - 4 * c
                        # diagonal block variant r: columns i < 128r are fully
                        # masked -> restrict all work to i in [off, 512).
                        off = 128 * r if r > 0 else 0
                        off_t[jb] = off
                        sc = psc_pool.tile([128, 1024], f32, tag="sc", name="sc")
                        kt_tile = qk_t[1, p, jb // 4]
                        q_tile = qk_t[0, p, c]
                        for par in range(2):
                            rows = slice(par * 64, par * 64 + 64)
                            nc.tensor.matmul(
                                sc[:, par * 512 + off:par * 512 + 512],
                                lhsT=kt_tile[rows, (jb % 4) * 128:
                                             (jb % 4) * 128 + 128],
                                rhs=q_tile[rows, off:512],
                                start=True, stop=(r < 0),
                                skip_group_check=True,
                            )
                            if r >= 0:
                                # additive causal mask on the diagonal
                                # 128-col window, accumulated on the PE so
                                # the AV matmul never waits on another
                                # engine beyond exp itself.
                                nc.tensor.matmul(
                                    sc[:, par * 512 + off:
                                       par * 512 + off + 128],
                                    lhsT=mask_sb[:, 0:128],
                                    rhs=mask_sb[:, 128:256],
                                    start=False, stop=True,
                                    skip_group_check=True,
                                )
                        ex = exp_pool.tile([128, 1024], bf16, tag="exp",
                                           name="ex")
                        ex_t[jb] = ex
                        sc2 = sc.rearrange("p (h i) -> p h i", h=2)
                        ex2 = ex.rearrange("p (h i) -> p h i", h=2)
                        nc.scalar.activation(ex2[:, :, off:512],
                                             sc2[:, :, off:512],
                                             Exp, scale=float(SCALE))
                        if DEBUG and (c, p, jb) == (0, 0, 0):
                            nc.sync.dma_start(dbg["dbg_ex0"], ex[:])
                            nc.sync.dma_start(dbg["dbg_q"], q_tile[:])
                            nc.sync.dma_start(dbg["dbg_k"], kt_tile[:])
                            nc.sync.dma_start(
                                dbg["dbg_vv"],
                                vv[:, 0].rearrange("p b c -> p (b c)"))
                        # pipelined: previous block's AV after this block's
                        # scores, so the PE never waits on exp(jb).
                        if jb > 0:
                            emit_av(jb - 1)
                        if jb == 0 and pending is not None:
                            pend_dn = emit_norm_a(pending)
                        if jb == 2 and pending is not None:
                            emit_norm_b(pending, pend_dn)
                            pending = None
                        # proj of chunk c-1 hosted in unit (c,0)'s last 4
                        # blocks (att normalized >= half a unit earlier)
                        pjb = min(njb - 4, 8)
                        if p == 0 and c > 0 and pjb <= jb < pjb + 4:
                            emit_proj(c - 1, 4 * (c - 1) + (jb - pjb),
                                      use_psc=(jb - pjb) % 2 == 1)
                        # sprinkle next chunk's v/q projections into unit
                        # (c,1) blocks (av(c,0)'s psum slot frees at jb3)
                        if p == 1 and c in (1, 2) and 4 <= jb <= 7:
                            if jb == 4:
                                spr_pv = pav_pool.tile([128, 1024], f32,
                                                       tag="av",
                                                       name=f"pv{c + 1}")
                            emit_v_group(c + 1, jb - 4, spr_pv)
                            if jb == 7:
                                emit_v_fill(c + 1, spr_pv)
                        if p == 1 and c == 2 and 8 <= jb <= 9:
                            if jb == 8:
                                spr_qp = psc_pool.tile([128, 1024], f32,
                                                       tag="sc", name="qp3")
                            emit_q_half(3, jb - 8, spr_qp)
                            if jb == 9:
                                for p_ in range(2):
                                    nc.vector.tensor_copy(
                                        qk_t[0, p_, 3][:],
                                        spr_qp[:, p_ * 512:p_ * 512 + 512])
                    emit_av(njb - 1)
                    pending = (c, p, av)

                # work not coverable by sprinkling (early short units)
                if c == 0:
                    emit_q(1)
                    emit_v(1)
                elif c == 1:
                    emit_q(2)

            pend_dn = emit_norm_a(pending)
            emit_norm_b(pending, pend_dn)
            for tt in range(12, 16):
                emit_proj(3, tt, use_psc=tt % 2 == 1)

    nc.compile()
    return nc


def _get_program():
    global _PROG
    if _PROG is None:
        _PROG = _build_program()
    return _PROG


def _host_inputs(x, Wqkv, Wproj):
    """Build the 8 per-core input maps."""
    bf = ml_dtypes.bfloat16
    x = np.asarray(x, np.float32)
    Wqkv = np.asarray(Wqkv, np.float32)
    Wproj = np.asarray(Wproj, np.float32)

    Wq = Wqkv[:, :D].reshape(D, H, DH)
    Wk = Wqkv[:, D:2 * D].reshape(D, H, DH)
    Wv = Wqkv[:, 2 * D:].reshape(D, H, DH)

    # mask tile: cols 0:128 = identity (lhsT of the additive-mask matmul),
    # cols 128:256 = additive causal bias (0 where j <= i, -30000 above the
    # diagonal; exp underflows to exact 0 after the 1/8 scale)
    j = np.arange(128)[:, None]
    i = np.arange(128)[None, :]
    ident = (j == i).astype(np.float32)
    mbias = np.where(j <= i, 0.0, -30000.0).astype(np.float32)
    mask = np.concatenate([ident, mbias], axis=1).astype(bf)  # [128, 256]

    # v-tile statics: per (jb, pair) block of VBS cols:
    # col 0 = 1 (odd sumexp), 1:64 zeros, 64:192 v data (zero-init),
    # col 192 = 1 (even sumexp), col 193 pad
    blk = np.zeros(VBS, np.float32)
    blk[0] = blk[192] = 1.0
    vinit = np.tile(blk, (128, NTT * 2)).astype(bf)

    in_maps = []
    for b in range(B):
        xT = np.ascontiguousarray(x[b].T).astype(bf)  # [D, T]
        for g in range(4):
            hs = slice(g * HPG, (g + 1) * HPG)
            # v columns pair-swapped: [h1, h0, h3, h2]
            vord = [g * HPG + 1, g * HPG + 0, g * HPG + 3, g * HPG + 2]
            wqkv = np.concatenate(
                [Wq[:, hs].reshape(D, HPG * DH),
                 Wk[:, hs].reshape(D, HPG * DH),
                 Wv[:, vord].reshape(D, HPG * DH)], axis=1,
            ).astype(bf)
            wp = (Wproj[g * 256:(g + 1) * 256]
                  .reshape(2, 128, D).transpose(1, 0, 2).reshape(128, 2 * D)
                  ).astype(bf)
            in_maps.append({
                "xT": np.ascontiguousarray(xT),
                "wqkv": np.ascontiguousarray(wqkv),
                "wproj": np.ascontiguousarray(wp),
                "mask": mask,
                "vinit": vinit,
                "ones": np.ones((128, 128), np.float32),
            })
    return in_maps


def kernel(x, Wqkv, Wproj):
    from concourse.bass_utils import run_bass_kernel_spmd

    nc = _get_program()
    in_maps = _host_inputs(x, Wqkv, Wproj)
    res = run_bass_kernel_spmd(nc, in_maps, core_ids=list(range(8)))
    outs = [r["out"] for r in res.results]
    full = np.stack(
        [outs[b * 4] + outs[b * 4 + 1] + outs[b * 4 + 2] + outs[b * 4 + 3]
         for b in range(B)]
    ).astype(np.float32)
    return full


# revision 32
# speedup vs baseline: 1.0289x; 1.0289x over previous
"""Causal self-attention (B=2, T=2048, D=1024, H=16) on 8 trn2 NeuronCores.

Sharding: core = b*4 + g  (b = batch 0/1, g = head-group of 4 heads).
Each core computes its 4 heads' attention for its batch plus the partial
output projection (Wproj rows for those heads); host sums the 4 partials
per batch (the tensor-parallel all-reduce).

All matmul operands are bf16 (psum accumulation stays f32): halves DMA
bytes, 1 cyc/row LDWEIGHTS, and much lower PE power than f32r (which
triggered hardware activity-throttling to ~50-70% util).

The attention inner loop is SOFTWARE-PIPELINED: the PE issue order per
key-block is  sc(jb) -> av(jb-1)  so the PE streams the next block's
scores while the Activation engine computes exp of the current block;
the old order (sc(jb) -> av(jb), in-order PE queue) stalled the PE on
the full sc->exp->mask->av chain (~2.2us/block).  Normalization of unit
(c,p) and the output projection of chunk c-1 are likewise deferred into
the early blocks of the next unit so their PE ops never stall.

Stage 1: per-kt DMA arrivals drive a kt-OUTER k-projection pass (8 psum
groups), so the PE works as soon as the first 128-row slice of xT/w
lands; the v pass runs from resident SBUF; q chunks are projected
inside the attention loop right before use.

Per-core layouts:
  xT      [1024, 2048]   x[b] transposed, bf16     -> sbuf xt  [128, 8*2048]
  wqkv    [1024, 768]    [q 4h | k 4h | v' 4h]     -> sbuf w   [128, 8*768]
          (v' columns pair-swapped: [h1 h0 h3 h2] so the psum v block is
           [odd | even] per pair, matching the v-tile copy layout)
  wproj   [128, 2048]    pair-major Wproj rows     -> sbuf     [128, 2*1024]
  out     [2048, 1024]   partial projection output (f32)

qT/kT pair tiles [128, 2048]: head-even rows 0:64, head-odd rows 64:128
(row-tiled K=64 matmuls use SBUF partitions 0-63 / 64-127).

v tiles: ONE tile vv [128, 16, 2, 194]; per (key-block jb, pair) block:
  [one | zeros*63 | v_o (64) | v_e (64) | one | pad]
  odd  AV lhsT = block[0:128]   -> sumexp row 0,  out rows 64:128
  even AV lhsT = block[128:193] -> out rows 0:64, sumexp row 64
  per-tt psum->sbuf fill is ONE DVE copy into block[64:192] (both pairs).

Softmax normalization: sumexp rows copied psum->sbuf as f32r (DVE),
broadcast to 128 partitions with K=1 ones-matmuls (f32r), fast DVE
reciprocal on the wide broadcast, one multiply per head half.
"""

import os
import numpy as np
import ml_dtypes

DEBUG = os.environ.get("ATTN_DEBUG", "0") == "1"

B, T, D, H, DH = 2, 2048, 1024, 16, 64
HPG = 4          # heads per group (per core)
NKT = D // 128   # 8 contraction tiles over D
NTT = T // 128   # 16 tiles over T (also key blocks)
NC_ = 4          # 4 i-chunks of 512 queries
VBS = 194        # v-tile per-(jb,pair) block: 64 static + 128 v + one + pad
SCALE = 1.0 / np.sqrt(DH)

_PROG = None


def _build_program():
    from contextlib import ExitStack
    from concourse import bacc, mybir, tile

    f32 = mybir.dt.float32
    f32r = mybir.dt.float32r
    bf16 = mybir.dt.bfloat16
    Exp = mybir.ActivationFunctionType.Exp

    nc = bacc.Bacc(
        "TRN2", target_bir_lowering=False, debug=False, enable_asserts=False,
        num_devices=8,
    )
    xT_d = nc.dram_tensor("xT", [D, T], bf16, kind="ExternalInput").ap()
    wqkv_d = nc.dram_tensor("wqkv", [D, 3 * HPG * DH], bf16, kind="ExternalInput").ap()
    wproj_d = nc.dram_tensor("wproj", [128, 2 * D], bf16, kind="ExternalInput").ap()
    mask_d = nc.dram_tensor("mask", [128, 256], bf16, kind="ExternalInput").ap()
    vinit_d = nc.dram_tensor("vinit", [128, NTT * 2 * VBS], bf16,
                             kind="ExternalInput").ap()
    ones_d = nc.dram_tensor("ones", [128, 128], f32r, kind="ExternalInput").ap()
    out_d = nc.dram_tensor("out", [T, D], f32, kind="ExternalOutput").ap()
    if DEBUG:
        dbg = {
            "dbg_q": nc.dram_tensor("dbg_q", [128, 512], bf16,
                                    kind="ExternalOutput").ap(),
            "dbg_k": nc.dram_tensor("dbg_k", [128, 512], bf16,
                                    kind="ExternalOutput").ap(),
            "dbg_ex0": nc.dram_tensor("dbg_ex0", [128, 1024], bf16,
                                      kind="ExternalOutput").ap(),
            "dbg_vv": nc.dram_tensor("dbg_vv", [128, 2 * VBS], bf16,
                                     kind="ExternalOutput").ap(),
            "dbg_av": nc.dram_tensor("dbg_av", [128, 1024], f32,
                                     kind="ExternalOutput").ap(),
            "dbg_dn": nc.dram_tensor("dbg_dn", [65, 512], f32,
                                     kind="ExternalOutput").ap(),
            "dbg_bcs": nc.dram_tensor("dbg_bcs", [128, 1024], f32,
                                      kind="ExternalOutput").ap(),
            "dbg_rc": nc.dram_tensor("dbg_rc", [128, 1024], f32,
                                     kind="ExternalOutput").ap(),
            "dbg_att": nc.dram_tensor("dbg_att", [128, 512], bf16,
                                      kind="ExternalOutput").ap(),
        }

    with tile.TileContext(nc) as tc, ExitStack() as ctx:
        # ---- persistent pools -------------------------------------------
        const_pool = ctx.enter_context(tc.tile_pool(name="const", bufs=1))
        qk_pool = ctx.enter_context(tc.tile_pool(name="qk", bufs=1))
        v_pool = ctx.enter_context(tc.tile_pool(name="v", bufs=1))

        mask_sb = const_pool.tile([128, 256], bf16, tag="mask")
        ones_sb = const_pool.tile([128, 128], f32r, tag="ones")
        wproj_sb = const_pool.tile([128, 2 * D], bf16, tag="wproj")

        # per-chunk q/k tiles: qk_t[(qk, p, c)] = [128, 512]
        qk_t = {}
        for qk in range(2):
            for p in range(2):
                for c in range(NC_):
                    qk_t[qk, p, c] = qk_pool.tile(
                        [128, 512], bf16, tag=f"qk{qk}{p}{c}",
                        name=f"qkt{qk}{p}{c}")
        # all v tiles in one [128, jb, pair, VBS] tile
        vv = v_pool.tile([128, NTT, 2, VBS], bf16, tag="vv", name="vv")

        # ---- stage 1: k + v projection ----------------------------------
        exp_pool = ctx.enter_context(tc.tile_pool(name="exp", bufs=8))
        xt_pool = ctx.enter_context(tc.tile_pool(name="xt", bufs=1))
        wq_pool = ctx.enter_context(tc.tile_pool(name="wq", bufs=1))
        xt_sb = xt_pool.tile([128, NKT * T], bf16, tag="xt")
        w_sb = wq_pool.tile([128, NKT * 768], bf16, tag="w")
        for kt in range(NKT):
            nc.sync.dma_start(
                xt_sb[:, kt * T:(kt + 1) * T],
                xT_d[kt * 128:(kt + 1) * 128, :]
            )
            nc.sync.dma_start(
                w_sb[:, kt * 768:(kt + 1) * 768],
                wqkv_d[kt * 128:(kt + 1) * 128, :],
            )
        nc.sync.dma_start(
            vv.rearrange("p a b c -> p (a b c)"), vinit_d[:])
        nc.sync.dma_start(mask_sb[:], mask_d[:])
        nc.sync.dma_start(ones_sb[:], ones_d[:])
        nc.sync.dma_start(wproj_sb[:], wproj_d[:])

        # ---- pass A: k projection, kt-OUTER so each DMA arrival feeds ---
        # 8 matmuls (one per (c,p) psum group) immediately.
        with tc.tile_pool(name="pk", bufs=1, space="PSUM") as pk_pool:
            pk = {}
            for c in range(NC_):
                for p in range(2):
                    pk[p, c] = pk_pool.tile([128, 512], f32, tag=f"pk{p}{c}",
                                            name=f"pk{p}{c}")
            for kt in range(NKT):
                for c in range(NC_):
                    for p in range(2):
                        nc.tensor.matmul(
                            pk[p, c][:],
                            lhsT=w_sb[:, kt * 768 + 256 + p * 128:
                                      kt * 768 + 256 + p * 128 + 128],
                            rhs=xt_sb[:, kt * T + c * 512:
                                      kt * T + c * 512 + 512],
                            start=(kt == 0), stop=(kt == NKT - 1),
                            skip_group_check=True,
                        )
            for c in range(NC_):
                for p in range(2):
                    nc.scalar.copy(qk_t[1, p, c][:], pk[p, c][:])

        # ---- stage 2+3: pipelined attention -----------------------------
        # (the v and q projections for chunk c are emitted inside the
        # chunk loop, so attention/exp work starts right after pass A and
        # the projection matmuls fill PE slack whenever the Activation
        # engine paces the per-block exp chain)
        psc_pool = ctx.enter_context(
            tc.tile_pool(name="psc", bufs=2, space="PSUM"))
        att_pool = ctx.enter_context(tc.tile_pool(name="att", bufs=1))
        att_t = {}
        for p in range(2):
            for c in range(NC_):
                att_t[p, c] = att_pool.tile([128, 512], bf16, tag=f"att{p}{c}",
                                            name=f"attt{p}{c}")

        with (
            tc.tile_pool(name="dn", bufs=2) as dn_pool,
            tc.tile_pool(name="rc", bufs=2) as rc_pool,
            tc.tile_pool(name="ot", bufs=2) as ot_pool,
            tc.tile_pool(name="pav", bufs=2, space="PSUM") as pav_pool,
        ):
            def emit_norm_a(pend):
                """Normalize stage a: copy the sumexp rows psum->sbuf."""
                av_ = pend[2]
                dn = dn_pool.tile([65, 512], f32r, tag="dn", name="dn")
                nc.vector.tensor_copy(dn[64:65, :], av_[64:65, 0:512])
                nc.vector.tensor_copy(dn[0:1, :], av_[0:1, 512:1024])
                return dn

            def emit_norm_b(pend, dn):
                """Normalize stage b: att = av / sumexp.

                The denominators are broadcast to 128 partitions with K=1
                ones-matmuls, staged psum->sbuf with a plain copy, and
                reciprocal'd from SBUF: the fast DVE reciprocal reads
                garbage from PSUM on real hardware (CoreSim models it
                fine), so it must only ever see SBUF inputs at base
                partition 0.
                """
                c_, p_, av_ = pend
                bc = psc_pool.tile([128, 1024], f32, tag="sc", name="bc")
                nc.tensor.matmul(bc[:, 0:512], lhsT=ones_sb[64:65, :],
                                 rhs=dn[64:65, :], start=True, stop=True)
                nc.tensor.matmul(bc[:, 512:1024], lhsT=ones_sb[0:1, :],
                                 rhs=dn[0:1, :], start=True, stop=True)
                bcs = rc_pool.tile([128, 1024], f32, tag="bcs", name="bcs")
                nc.vector.tensor_copy(bcs[0:64, :], bc[0:64, :])
                rc = rc_pool.tile([128, 1024], f32, tag="rc", name="rc")
                nc.vector.reciprocal_approx_fast(rc[0:64, :], bcs[0:64, :])
                # move the odd-half reciprocals to partitions 64:128 (DMA is
                # the only partition-crossing mover outside the PE)
                nc.sync.dma_start(rc[64:128, 0:512], rc[0:64, 512:1024])
                nc.vector.tensor_mul(
                    att_t[p_, c_][0:64, :], av_[0:64, 0:512], rc[0:64, 0:512])
                nc.vector.tensor_mul(
                    att_t[p_, c_][64:128, :], av_[64:128, 512:1024],
                    rc[64:128, 0:512])
                if DEBUG and (c_, p_) == (0, 0):
                    ava = dn_pool.tile([128, 1024], f32, tag="ava", name="ava")
                    nc.vector.tensor_copy(ava[:], av_[:])
                    nc.sync.dma_start(dbg["dbg_av"], ava[:])
                    nc.sync.dma_start(dbg["dbg_dn"][0:65, :], dn.bitcast(f32))
                    nc.sync.dma_start(dbg["dbg_rc"], rc[:])
                    nc.sync.dma_start(dbg["dbg_att"], att_t[0, 0][:])

            def emit_proj(c_, tt, use_psc):
                """Project output t-tile tt from att chunk c_."""
                pool = psc_pool if use_psc else pav_pool
                pp = pool.tile([128, 1024], f32, tag="sc" if use_psc else "av",
                               name="pp")
                ot = ot_pool.tile([128, D], f32, tag="ot", name="ot")
                for ch in range(2):
                    for p in range(2):
                        nc.tensor.matmul(
                            pp[:, ch * 512:ch * 512 + 512],
                            lhsT=att_t[p, tt // 4][:, (tt % 4) * 128:
                                                   (tt % 4) * 128 + 128],
                            rhs=wproj_sb[:, p * D + ch * 512:
                                         p * D + ch * 512 + 512],
                            start=(p == 0), stop=(p == 1),
                        )
                    nc.vector.tensor_copy(
                        ot[:, ch * 512:ch * 512 + 512],
                        pp[:, ch * 512:ch * 512 + 512])
                    nc.sync.dma_start(
                        out_d[tt * 128:tt * 128 + 128,
                              ch * 512:ch * 512 + 512],
                        ot[:, ch * 512:ch * 512 + 512])

            def emit_v(c_):
                """v projection for chunk c_'s 4 key blocks (one psum tile
                holds all 4 tt groups; single DVE fill copy into vv)."""
                pv = pav_pool.tile([128, 1024], f32, tag="av", name=f"pv{c_}")
                for ttl in range(4):
                    emit_v_group(c_, ttl, pv)
                emit_v_fill(c_, pv)

            def emit_v_group(c_, ttl, pv):
                tt = 4 * c_ + ttl
                for kt in range(NKT):
                    nc.tensor.matmul(
                        pv[:, ttl * 256:ttl * 256 + 256],
                        lhsT=xt_sb[:, kt * T + tt * 128:
                                   kt * T + tt * 128 + 128],
                        rhs=w_sb[:, kt * 768 + 512:kt * 768 + 768],
                        start=(kt == 0), stop=(kt == NKT - 1),
                        skip_group_check=True,
                    )

            def emit_v_fill(c_, pv):
                pv4 = pv.rearrange("p (t pr c) -> p t pr c", t=4, pr=2)
                nc.vector.tensor_copy(
                    vv[:, 4 * c_:4 * c_ + 4, :, 64:192], pv4[:])

            def emit_q_half(c_, p_, qp):
                for kt in range(NKT):
                    nc.tensor.matmul(
                        qp[:, p_ * 512:p_ * 512 + 512],
                        lhsT=w_sb[:, kt * 768 + p_ * 128:
                                  kt * 768 + p_ * 128 + 128],
                        rhs=xt_sb[:, kt * T + c_ * 512:kt * T + c_ * 512 + 512],
                        start=(kt == 0), stop=(kt == NKT - 1),
                        skip_group_check=True,
                    )

            def emit_q(c_):
                qp = psc_pool.tile([128, 1024], f32, tag="sc", name=f"qp{c_}")
                for p_ in range(2):
                    emit_q_half(c_, p_, qp)
                for p_ in range(2):
                    nc.vector.tensor_copy(qk_t[0, p_, c_][:],
                                          qp[:, p_ * 512:p_ * 512 + 512])

            pending = None  # (c, p, av tile) awaiting normalization
            # sprinkle state: v/q work for chunk c+1 emitted between the
            # blocks of unit (c,1) so the PE has work whenever the
            # Activation engine paces the exp chain.
            spr_pv = None
            for c in range(NC_):
                if c == 0:
                    emit_v(0)
                    emit_q(0)

                for p in range(2):
                    av = pav_pool.tile([128, 1024], f32, tag="av", name="av")
                    njb = 4 * c + 4
                    ex_t = [None] * njb
                    off_t = [0] * njb

                    def emit_av(jb):
                        o = off_t[jb]
                        nc.tensor.matmul(
                            av[0:65, o:512],
                            lhsT=vv[:, jb, p, 128:193],
                            rhs=ex_t[jb][:, o:512],
                            start=(jb == 0), stop=(jb == njb - 1),
                            skip_group_check=True,
                        )
                        nc.tensor.matmul(
                            av[:, 512 + o:1024],
                            lhsT=vv[:, jb, p, 0:128],
                            rhs=ex_t[jb][:, 512 + o:1024],
                            start=(jb == 0), stop=(jb == njb - 1),
                            skip_group_check=True,
                        )

                    for jb in range(njb):
                        r = jb - 4 * c
                        # diagonal block variant r: columns i < 128r are fully
                        # masked -> restrict all work to i in [off, 512).
                        off = 128 * r if r > 0 else 0
                        off_t[jb] = off
                        sc = psc_pool.tile([128, 1024], f32, tag="sc", name="sc")
                        kt_tile = qk_t[1, p, jb // 4]
                        q_tile = qk_t[0, p, c]
                        for par in range(2):
                            rows = slice(par * 64, par * 64 + 64)
                            nc.tensor.matmul(
                                sc[:, par * 512 + off:par * 512 + 512],
                                lhsT=kt_tile[rows, (jb % 4) * 128:
                                             (jb % 4) * 128 + 128],
                                rhs=q_tile[rows, off:512],
                                start=True, stop=True,
                            )
                        ex = exp_pool.tile([128, 1024], bf16, tag="exp",
                                           name="ex")
                        ex_t[jb] = ex
                        sc2 = sc.rearrange("p (h i) -> p h i", h=2)
                        ex2 = ex.rearrange("p (h i) -> p h i", h=2)
                        nc.scalar.activation(ex2[:, :, off:512],
                                             sc2[:, :, off:512],
                                             Exp, scale=float(SCALE))
                        if r >= 0:
                            # only the [off, off+128) columns straddle the
                            # diagonal; beyond them the block is fully valid.
                            mk2 = mask_sb.rearrange("p (h i) -> p h i", h=2)
                            nc.gpsimd.tensor_mul(
                                ex2[:, :, off:off + 128],
                                ex2[:, :, off:off + 128], mk2[:],
                            )
                        if DEBUG and (c, p, jb) == (0, 0, 0):
                            nc.sync.dma_start(dbg["dbg_ex0"], ex[:])
                            nc.sync.dma_start(dbg["dbg_q"], q_tile[:])
                            nc.sync.dma_start(dbg["dbg_k"], kt_tile[:])
                            nc.sync.dma_start(
                                dbg["dbg_vv"],
                                vv[:, 0].rearrange("p b c -> p (b c)"))
                        # pipelined: previous block's AV after this block's
                        # scores, so the PE never waits on exp(jb).
                        if jb > 0:
                            emit_av(jb - 1)
                        if jb == 1 and pending is not None:
                            pend_dn = emit_norm_a(pending)
                        if jb == 3 and pending is not None:
                            emit_norm_b(pending, pend_dn)
                            pending = None
                        # proj of chunk c-1 hosted in unit (c,0)'s last 4
                        # blocks (att normalized >= half a unit earlier)
                        if p == 0 and c > 0 and njb - 4 <= jb:
                            emit_proj(c - 1, 4 * (c - 1) + (jb - (njb - 4)),
                                      use_psc=(jb - (njb - 4)) % 2 == 1)
                        # sprinkle next chunk's v/q projections into unit
                        # (c,1) blocks (av(c,0)'s psum slot frees at jb3)
                        if p == 1 and c in (1, 2) and 4 <= jb <= 7:
                            if jb == 4:
                                spr_pv = pav_pool.tile([128, 1024], f32,
                                                       tag="av",
                                                       name=f"pv{c + 1}")
                            emit_v_group(c + 1, jb - 4, spr_pv)
                            if jb == 7:
                                emit_v_fill(c + 1, spr_pv)
                        if p == 1 and c == 2 and 8 <= jb <= 9:
                            if jb == 8:
                                spr_qp = psc_pool.tile([128, 1024], f32,
                                                       tag="sc", name="qp3")
                            emit_q_half(3, jb - 8, spr_qp)
                            if jb == 9:
                                for p_ in range(2):
                                    nc.vector.tensor_copy(
                                        qk_t[0, p_, 3][:],
                                        spr_qp[:, p_ * 512:p_ * 512 + 512])
                    emit_av(njb - 1)
                    pending = (c, p, av)

                # work not coverable by sprinkling (early short units)
                if c == 0:
                    emit_v(1)
                    emit_q(1)
                elif c == 1:
                    emit_q(2)

            pend_dn = emit_norm_a(pending)
            emit_norm_b(pending, pend_dn)
            for tt in range(12, 16):
                emit_proj(3, tt, use_psc=tt % 2 == 1)

    nc.compile()
    return nc


def _get_program():
    global _PROG
    if _PROG is None:
        _PROG = _build_program()
    return _PROG


def _host_inputs(x, Wqkv, Wproj):
    """Build the 8 per-core input maps."""
    bf = ml_dtypes.bfloat16
    x = np.asarray(x, np.float32)
    Wqkv = np.asarray(Wqkv, np.float32)
    Wproj = np.asarray(Wproj, np.float32)

    Wq = Wqkv[:, :D].reshape(D, H, DH)
    Wk = Wqkv[:, D:2 * D].reshape(D, H, DH)
    Wv = Wqkv[:, 2 * D:].reshape(D, H, DH)

    # causal mask: keeps j <= i ; duplicated to 256 cols
    j = np.arange(128)[:, None]
    i = np.arange(128)[None, :]
    tri = (j <= i).astype(np.float32)
    mask = np.concatenate([tri, tri], axis=1).astype(bf)  # [128, 256]

    # v-tile statics: per (jb, pair) block of VBS cols:
    # col 0 = 1 (odd sumexp), 1:64 zeros, 64:192 v data (zero-init),
    # col 192 = 1 (even sumexp), col 193 pad
    blk = np.zeros(VBS, np.float32)
    blk[0] = blk[192] = 1.0
    vinit = np.tile(blk, (128, NTT * 2)).astype(bf)

    in_maps = []
    for b in range(B):
        xT = np.ascontiguousarray(x[b].T).astype(bf)  # [D, T]
        for g in range(4):
            hs = slice(g * HPG, (g + 1) * HPG)
            # v columns pair-swapped: [h1, h0, h3, h2]
            vord = [g * HPG + 1, g * HPG + 0, g * HPG + 3, g * HPG + 2]
            wqkv = np.concatenate(
                [Wq[:, hs].reshape(D, HPG * DH),
                 Wk[:, hs].reshape(D, HPG * DH),
                 Wv[:, vord].reshape(D, HPG * DH)], axis=1,
            ).astype(bf)
            wp = (Wproj[g * 256:(g + 1) * 256]
                  .reshape(2, 128, D).transpose(1, 0, 2).reshape(128, 2 * D)
                  ).astype(bf)
            in_maps.append({
                "xT": np.ascontiguousarray(xT),
                "wqkv": np.ascontiguousarray(wqkv),
                "wproj": np.ascontiguousarray(wp),
                "mask": mask,
                "vinit": vinit,
                "ones": np.ones((128, 128), np.float32),
            })
    return in_maps


def kernel(x, Wqkv, Wproj):
    from concourse.bass_utils import run_bass_kernel_spmd

    nc = _get_program()
    in_maps = _host_inputs(x, Wqkv, Wproj)
    res = run_bass_kernel_spmd(nc, in_maps, core_ids=list(range(8)))
    outs = [r["out"] for r in res.results]
    full = np.stack(
        [outs[b * 4] + outs[b * 4 + 1] + outs[b * 4 + 2] + outs[b * 4 + 3]
         for b in range(B)]
    ).astype(np.float32)
    return full


# revision 33
# speedup vs baseline: 1.0358x; 1.0066x over previous
"""Causal self-attention (B=2, T=2048, D=1024, H=16) on 8 trn2 NeuronCores.

Sharding: core = b*4 + g  (b = batch 0/1, g = head-group of 4 heads).
Each core computes its 4 heads' attention for its batch plus the partial
output projection (Wproj rows for those heads); host sums the 4 partials
per batch (the tensor-parallel all-reduce).

All matmul operands are bf16 (psum accumulation stays f32): halves DMA
bytes, 1 cyc/row LDWEIGHTS, and much lower PE power than f32r (which
triggered hardware activity-throttling to ~50-70% util).

The attention inner loop is SOFTWARE-PIPELINED: the PE issue order per
key-block is  sc(jb) -> av(jb-1)  so the PE streams the next block's
scores while the Activation engine computes exp of the current block;
the old order (sc(jb) -> av(jb), in-order PE queue) stalled the PE on
the full sc->exp->mask->av chain (~2.2us/block).  Normalization of unit
(c,p) and the output projection of chunk c-1 are likewise deferred into
the early blocks of the next unit so their PE ops never stall.

Stage 1: per-kt DMA arrivals drive a kt-OUTER k-projection pass (8 psum
groups), so the PE works as soon as the first 128-row slice of xT/w
lands; the v pass runs from resident SBUF; q chunks are projected
inside the attention loop right before use.

Per-core layouts:
  xT      [1024, 2048]   x[b] transposed, bf16     -> sbuf xt  [128, 8*2048]
  wqkv    [1024, 768]    [q 4h | k 4h | v' 4h]     -> sbuf w   [128, 8*768]
          (v' columns pair-swapped: [h1 h0 h3 h2] so the psum v block is
           [odd | even] per pair, matching the v-tile copy layout)
  wproj   [128, 2048]    pair-major Wproj rows     -> sbuf     [128, 2*1024]
  out     [2048, 1024]   partial projection output (f32)

qT/kT pair tiles [128, 2048]: head-even rows 0:64, head-odd rows 64:128
(row-tiled K=64 matmuls use SBUF partitions 0-63 / 64-127).

v tiles: ONE tile vv [128, 16, 2, 194]; per (key-block jb, pair) block:
  [one | zeros*63 | v_o (64) | v_e (64) | one | pad]
  odd  AV lhsT = block[0:128]   -> sumexp row 0,  out rows 64:128
  even AV lhsT = block[128:193] -> out rows 0:64, sumexp row 64
  per-tt psum->sbuf fill is ONE DVE copy into block[64:192] (both pairs).

Softmax normalization: sumexp rows copied psum->sbuf as f32r (DVE),
broadcast to 128 partitions with K=1 ones-matmuls (f32r), fast DVE
reciprocal on the wide broadcast, one multiply per head half.
"""

import os
import numpy as np
import ml_dtypes

DEBUG = os.environ.get("ATTN_DEBUG", "0") == "1"

B, T, D, H, DH = 2, 2048, 1024, 16, 64
HPG = 4          # heads per group (per core)
NKT = D // 128   # 8 contraction tiles over D
NTT = T // 128   # 16 tiles over T (also key blocks)
NC_ = 4          # 4 i-chunks of 512 queries
VBS = 194        # v-tile per-(jb,pair) block: 64 static + 128 v + one + pad
SCALE = 1.0 / np.sqrt(DH)

_PROG = None


def _build_program():
    from contextlib import ExitStack
    from concourse import bacc, mybir, tile

    f32 = mybir.dt.float32
    f32r = mybir.dt.float32r
    bf16 = mybir.dt.bfloat16
    Exp = mybir.ActivationFunctionType.Exp

    nc = bacc.Bacc(
        "TRN2", target_bir_lowering=False, debug=False, enable_asserts=False,
        num_devices=8,
    )
    xT_d = nc.dram_tensor("xT", [D, T], bf16, kind="ExternalInput").ap()
    wqkv_d = nc.dram_tensor("wqkv", [D, 3 * HPG * DH], bf16, kind="ExternalInput").ap()
    wproj_d = nc.dram_tensor("wproj", [128, 2 * D], bf16, kind="ExternalInput").ap()
    mask_d = nc.dram_tensor("mask", [128, 256], bf16, kind="ExternalInput").ap()
    vinit_d = nc.dram_tensor("vinit", [128, NTT * 2 * VBS], bf16,
                             kind="ExternalInput").ap()
    ones_d = nc.dram_tensor("ones", [128, 128], f32r, kind="ExternalInput").ap()
    out_d = nc.dram_tensor("out", [T, D], f32, kind="ExternalOutput").ap()
    if DEBUG:
        dbg = {
            "dbg_q": nc.dram_tensor("dbg_q", [128, 512], bf16,
                                    kind="ExternalOutput").ap(),
            "dbg_k": nc.dram_tensor("dbg_k", [128, 512], bf16,
                                    kind="ExternalOutput").ap(),
            "dbg_ex0": nc.dram_tensor("dbg_ex0", [128, 1024], bf16,
                                      kind="ExternalOutput").ap(),
            "dbg_vv": nc.dram_tensor("dbg_vv", [128, 2 * VBS], bf16,
                                     kind="ExternalOutput").ap(),
            "dbg_av": nc.dram_tensor("dbg_av", [128, 1024], f32,
                                     kind="ExternalOutput").ap(),
            "dbg_dn": nc.dram_tensor("dbg_dn", [65, 512], f32,
                                     kind="ExternalOutput").ap(),
            "dbg_bcs": nc.dram_tensor("dbg_bcs", [128, 1024], f32,
                                      kind="ExternalOutput").ap(),
            "dbg_rc": nc.dram_tensor("dbg_rc", [128, 1024], f32,
                                     kind="ExternalOutput").ap(),
            "dbg_att": nc.dram_tensor("dbg_att", [128, 512], bf16,
                                      kind="ExternalOutput").ap(),
        }

    with tile.TileContext(nc) as tc, ExitStack() as ctx:
        # ---- persistent pools -------------------------------------------
        const_pool = ctx.enter_context(tc.tile_pool(name="const", bufs=1))
        qk_pool = ctx.enter_context(tc.tile_pool(name="qk", bufs=1))
        v_pool = ctx.enter_context(tc.tile_pool(name="v", bufs=1))

        mask_sb = const_pool.tile([128, 256], bf16, tag="mask")
        ones_sb = const_pool.tile([128, 128], f32r, tag="ones")
        wproj_sb = const_pool.tile([128, 2 * D], bf16, tag="wproj")

        # per-chunk q/k tiles: qk_t[(qk, p, c)] = [128, 512]
        qk_t = {}
        for qk in range(2):
            for p in range(2):
                for c in range(NC_):
                    qk_t[qk, p, c] = qk_pool.tile(
                        [128, 512], bf16, tag=f"qk{qk}{p}{c}",
                        name=f"qkt{qk}{p}{c}")
        # all v tiles in one [128, jb, pair, VBS] tile
        vv = v_pool.tile([128, NTT, 2, VBS], bf16, tag="vv", name="vv")

        # ---- stage 1: k + v projection ----------------------------------
        exp_pool = ctx.enter_context(tc.tile_pool(name="exp", bufs=8))
        xt_pool = ctx.enter_context(tc.tile_pool(name="xt", bufs=1))
        wq_pool = ctx.enter_context(tc.tile_pool(name="wq", bufs=1))
        xt_sb = xt_pool.tile([128, NKT * T], bf16, tag="xt")
        w_sb = wq_pool.tile([128, NKT * 768], bf16, tag="w")
        for kt in range(NKT):
            nc.sync.dma_start(
                xt_sb[:, kt * T:(kt + 1) * T],
                xT_d[kt * 128:(kt + 1) * 128, :]
            )
            nc.sync.dma_start(
                w_sb[:, kt * 768:(kt + 1) * 768],
                wqkv_d[kt * 128:(kt + 1) * 128, :],
            )
        nc.sync.dma_start(
            vv.rearrange("p a b c -> p (a b c)"), vinit_d[:])
        nc.sync.dma_start(mask_sb[:], mask_d[:])
        nc.sync.dma_start(ones_sb[:], ones_d[:])
        nc.sync.dma_start(wproj_sb[:], wproj_d[:])

        # ---- pass A: k projection, kt-OUTER so each DMA arrival feeds ---
        # 8 matmuls (one per (c,p) psum group) immediately.
        with tc.tile_pool(name="pk", bufs=1, space="PSUM") as pk_pool:
            pk = {}
            for c in range(NC_):
                for p in range(2):
                    pk[p, c] = pk_pool.tile([128, 512], f32, tag=f"pk{p}{c}",
                                            name=f"pk{p}{c}")
            for kt in range(NKT):
                for c in range(NC_):
                    for p in range(2):
                        nc.tensor.matmul(
                            pk[p, c][:],
                            lhsT=w_sb[:, kt * 768 + 256 + p * 128:
                                      kt * 768 + 256 + p * 128 + 128],
                            rhs=xt_sb[:, kt * T + c * 512:
                                      kt * T + c * 512 + 512],
                            start=(kt == 0), stop=(kt == NKT - 1),
                            skip_group_check=True,
                        )
            for c in range(NC_):
                for p in range(2):
                    # alternate engines: the pass-A psum banks gate the
                    # first v matmuls, so drain them twice as fast
                    if (c + p) % 2 == 0:
                        nc.scalar.copy(qk_t[1, p, c][:], pk[p, c][:])
                    else:
                        nc.vector.tensor_copy(qk_t[1, p, c][:], pk[p, c][:])

        # ---- stage 2+3: pipelined attention -----------------------------
        # (the v and q projections for chunk c are emitted inside the
        # chunk loop, so attention/exp work starts right after pass A and
        # the projection matmuls fill PE slack whenever the Activation
        # engine paces the per-block exp chain)
        psc_pool = ctx.enter_context(
            tc.tile_pool(name="psc", bufs=2, space="PSUM"))
        att_pool = ctx.enter_context(tc.tile_pool(name="att", bufs=1))
        att_t = {}
        for p in range(2):
            for c in range(NC_):
                att_t[p, c] = att_pool.tile([128, 512], bf16, tag=f"att{p}{c}",
                                            name=f"attt{p}{c}")

        with (
            tc.tile_pool(name="dn", bufs=2) as dn_pool,
            tc.tile_pool(name="rc", bufs=2) as rc_pool,
            tc.tile_pool(name="ot", bufs=2) as ot_pool,
            tc.tile_pool(name="pav", bufs=2, space="PSUM") as pav_pool,
        ):
            def emit_norm_a(pend):
                """Normalize stage a: copy the sumexp rows psum->sbuf."""
                av_ = pend[2]
                dn = dn_pool.tile([65, 512], f32r, tag="dn", name="dn")
                nc.vector.tensor_copy(dn[64:65, :], av_[64:65, 0:512])
                nc.vector.tensor_copy(dn[0:1, :], av_[0:1, 512:1024])
                return dn

            def emit_norm_b(pend, dn):
                """Normalize stage b: att = av / sumexp.

                The denominators are broadcast to 128 partitions with K=1
                ones-matmuls, staged psum->sbuf with a plain copy, and
                reciprocal'd from SBUF: the fast DVE reciprocal reads
                garbage from PSUM on real hardware (CoreSim models it
                fine), so it must only ever see SBUF inputs at base
                partition 0.
                """
                c_, p_, av_ = pend
                bc = psc_pool.tile([128, 1024], f32, tag="sc", name="bc")
                nc.tensor.matmul(bc[:, 0:512], lhsT=ones_sb[64:65, :],
                                 rhs=dn[64:65, :], start=True, stop=True)
                nc.tensor.matmul(bc[:, 512:1024], lhsT=ones_sb[0:1, :],
                                 rhs=dn[0:1, :], start=True, stop=True)
                bcs = rc_pool.tile([128, 1024], f32, tag="bcs", name="bcs")
                nc.vector.tensor_copy(bcs[0:64, :], bc[0:64, :])
                rc = rc_pool.tile([128, 1024], f32, tag="rc", name="rc")
                nc.vector.reciprocal_approx_fast(rc[0:64, :], bcs[0:64, :])
                # move the odd-half reciprocals to partitions 64:128 (DMA is
                # the only partition-crossing mover outside the PE)
                nc.sync.dma_start(rc[64:128, 0:512], rc[0:64, 512:1024])
                nc.vector.tensor_mul(
                    att_t[p_, c_][0:64, :], av_[0:64, 0:512], rc[0:64, 0:512])
                nc.vector.tensor_mul(
                    att_t[p_, c_][64:128, :], av_[64:128, 512:1024],
                    rc[64:128, 0:512])
                if DEBUG and (c_, p_) == (0, 0):
                    ava = dn_pool.tile([128, 1024], f32, tag="ava", name="ava")
                    nc.vector.tensor_copy(ava[:], av_[:])
                    nc.sync.dma_start(dbg["dbg_av"], ava[:])
                    nc.sync.dma_start(dbg["dbg_dn"][0:65, :], dn.bitcast(f32))
                    nc.sync.dma_start(dbg["dbg_rc"], rc[:])
                    nc.sync.dma_start(dbg["dbg_att"], att_t[0, 0][:])

            def emit_proj(c_, tt, use_psc):
                """Project output t-tile tt from att chunk c_."""
                pool = psc_pool if use_psc else pav_pool
                pp = pool.tile([128, 1024], f32, tag="sc" if use_psc else "av",
                               name="pp")
                ot = ot_pool.tile([128, D], f32, tag="ot", name="ot")
                for ch in range(2):
                    for p in range(2):
                        nc.tensor.matmul(
                            pp[:, ch * 512:ch * 512 + 512],
                            lhsT=att_t[p, tt // 4][:, (tt % 4) * 128:
                                                   (tt % 4) * 128 + 128],
                            rhs=wproj_sb[:, p * D + ch * 512:
                                         p * D + ch * 512 + 512],
                            start=(p == 0), stop=(p == 1),
                        )
                    nc.vector.tensor_copy(
                        ot[:, ch * 512:ch * 512 + 512],
                        pp[:, ch * 512:ch * 512 + 512])
                    nc.sync.dma_start(
                        out_d[tt * 128:tt * 128 + 128,
                              ch * 512:ch * 512 + 512],
                        ot[:, ch * 512:ch * 512 + 512])

            def emit_v(c_):
                """v projection for chunk c_'s 4 key blocks (one psum tile
                holds all 4 tt groups; single DVE fill copy into vv)."""
                pv = pav_pool.tile([128, 1024], f32, tag="av", name=f"pv{c_}")
                for ttl in range(4):
                    emit_v_group(c_, ttl, pv)
                emit_v_fill(c_, pv)

            def emit_v_group(c_, ttl, pv):
                tt = 4 * c_ + ttl
                for kt in range(NKT):
                    nc.tensor.matmul(
                        pv[:, ttl * 256:ttl * 256 + 256],
                        lhsT=xt_sb[:, kt * T + tt * 128:
                                   kt * T + tt * 128 + 128],
                        rhs=w_sb[:, kt * 768 + 512:kt * 768 + 768],
                        start=(kt == 0), stop=(kt == NKT - 1),
                        skip_group_check=True,
                    )

            def emit_v_fill(c_, pv):
                pv4 = pv.rearrange("p (t pr c) -> p t pr c", t=4, pr=2)
                nc.vector.tensor_copy(
                    vv[:, 4 * c_:4 * c_ + 4, :, 64:192], pv4[:])

            def emit_q_half(c_, p_, qp):
                for kt in range(NKT):
                    nc.tensor.matmul(
                        qp[:, p_ * 512:p_ * 512 + 512],
                        lhsT=w_sb[:, kt * 768 + p_ * 128:
                                  kt * 768 + p_ * 128 + 128],
                        rhs=xt_sb[:, kt * T + c_ * 512:kt * T + c_ * 512 + 512],
                        start=(kt == 0), stop=(kt == NKT - 1),
                        skip_group_check=True,
                    )

            def emit_q(c_):
                qp = psc_pool.tile([128, 1024], f32, tag="sc", name=f"qp{c_}")
                for p_ in range(2):
                    emit_q_half(c_, p_, qp)
                for p_ in range(2):
                    nc.vector.tensor_copy(qk_t[0, p_, c_][:],
                                          qp[:, p_ * 512:p_ * 512 + 512])

            pending = None  # (c, p, av tile) awaiting normalization
            # sprinkle state: v/q work for chunk c+1 emitted between the
            # blocks of unit (c,1) so the PE has work whenever the
            # Activation engine paces the exp chain.
            spr_pv = None
            for c in range(NC_):
                if c == 0:
                    emit_v(0)
                    emit_q(0)

                for p in range(2):
                    av = pav_pool.tile([128, 1024], f32, tag="av", name="av")
                    njb = 4 * c + 4
                    ex_t = [None] * njb
                    off_t = [0] * njb

                    def emit_av(jb):
                        o = off_t[jb]
                        nc.tensor.matmul(
                            av[0:65, o:512],
                            lhsT=vv[:, jb, p, 128:193],
                            rhs=ex_t[jb][:, o:512],
                            start=(jb == 0), stop=(jb == njb - 1),
                            skip_group_check=True,
                        )
                        nc.tensor.matmul(
                            av[:, 512 + o:1024],
                            lhsT=vv[:, jb, p, 0:128],
                            rhs=ex_t[jb][:, 512 + o:1024],
                            start=(jb == 0), stop=(jb == njb - 1),
                            skip_group_check=True,
                        )

                    for jb in range(njb):
                        r = jb - 4 * c
                        # diagonal block variant r: columns i < 128r are fully
                        # masked -> restrict all work to i in [off, 512).
                        off = 128 * r if r > 0 else 0
                        off_t[jb] = off
                        sc = psc_pool.tile([128, 1024], f32, tag="sc", name="sc")
                        kt_tile = qk_t[1, p, jb // 4]
                        q_tile = qk_t[0, p, c]
                        for par in range(2):
                            rows = slice(par * 64, par * 64 + 64)
                            nc.tensor.matmul(
                                sc[:, par * 512 + off:par * 512 + 512],
                                lhsT=kt_tile[rows, (jb % 4) * 128:
                                             (jb % 4) * 128 + 128],
                                rhs=q_tile[rows, off:512],
                                start=True, stop=True,
                            )
                        ex = exp_pool.tile([128, 1024], bf16, tag="exp",
                                           name="ex")
                        ex_t[jb] = ex
                        sc2 = sc.rearrange("p (h i) -> p h i", h=2)
                        ex2 = ex.rearrange("p (h i) -> p h i", h=2)
                        nc.scalar.activation(ex2[:, :, off:512],
                                             sc2[:, :, off:512],
                                             Exp, scale=float(SCALE))
                        if r >= 0:
                            # only the [off, off+128) columns straddle the
                            # diagonal; beyond them the block is fully valid.
                            mk2 = mask_sb.rearrange("p (h i) -> p h i", h=2)
                            nc.gpsimd.tensor_mul(
                                ex2[:, :, off:off + 128],
                                ex2[:, :, off:off + 128], mk2[:],
                            )
                        if DEBUG and (c, p, jb) == (0, 0, 0):
                            nc.sync.dma_start(dbg["dbg_ex0"], ex[:])
                            nc.sync.dma_start(dbg["dbg_q"], q_tile[:])
                            nc.sync.dma_start(dbg["dbg_k"], kt_tile[:])
                            nc.sync.dma_start(
                                dbg["dbg_vv"],
                                vv[:, 0].rearrange("p b c -> p (b c)"))
                        # pipelined 2 deep: AV of block jb-2 after this
                        # block's scores, so the PE rides out exp jitter.
                        if jb > 1:
                            emit_av(jb - 2)
                        if jb == 1 and pending is not None:
                            pend_dn = emit_norm_a(pending)
                        if jb == 3 and pending is not None:
                            emit_norm_b(pending, pend_dn)
                            pending = None
                        # proj of chunk c-1 hosted in unit (c,0)'s last 4
                        # blocks (att normalized >= half a unit earlier)
                        if p == 0 and c > 0 and njb - 4 <= jb:
                            emit_proj(c - 1, 4 * (c - 1) + (jb - (njb - 4)),
                                      use_psc=(jb - (njb - 4)) % 2 == 1)
                        # sprinkle next chunk's v/q projections into unit
                        # (c,1) blocks (av(c,0)'s psum slot frees at jb3)
                        if p == 1 and c in (1, 2) and 4 <= jb <= 7:
                            if jb == 4:
                                spr_pv = pav_pool.tile([128, 1024], f32,
                                                       tag="av",
                                                       name=f"pv{c + 1}")
                            emit_v_group(c + 1, jb - 4, spr_pv)
                            if jb == 7:
                                emit_v_fill(c + 1, spr_pv)
                        if p == 1 and c == 2 and 8 <= jb <= 9:
                            if jb == 8:
                                spr_qp = psc_pool.tile([128, 1024], f32,
                                                       tag="sc", name="qp3")
                            emit_q_half(3, jb - 8, spr_qp)
                            if jb == 9:
                                for p_ in range(2):
                                    nc.vector.tensor_copy(
                                        qk_t[0, p_, 3][:],
                                        spr_qp[:, p_ * 512:p_ * 512 + 512])
                    emit_av(njb - 2)
                    emit_av(njb - 1)
                    pending = (c, p, av)

                # work not coverable by sprinkling (early short units)
                if c == 0:
                    emit_v(1)
                    emit_q(1)
                elif c == 1:
                    emit_q(2)

            pend_dn = emit_norm_a(pending)
            emit_norm_b(pending, pend_dn)
            for tt in range(12, 16):
                emit_proj(3, tt, use_psc=tt % 2 == 1)

    nc.compile()
    return nc


def _get_program():
    global _PROG
    if _PROG is None:
        _PROG = _build_program()
    return _PROG


def _host_inputs(x, Wqkv, Wproj):
    """Build the 8 per-core input maps."""
    bf = ml_dtypes.bfloat16
    x = np.asarray(x, np.float32)
    Wqkv = np.asarray(Wqkv, np.float32)
    Wproj = np.asarray(Wproj, np.float32)

    Wq = Wqkv[:, :D].reshape(D, H, DH)
    Wk = Wqkv[:, D:2 * D].reshape(D, H, DH)
    Wv = Wqkv[:, 2 * D:].reshape(D, H, DH)

    # causal mask: keeps j <= i ; duplicated to 256 cols
    j = np.arange(128)[:, None]
    i = np.arange(128)[None, :]
    tri = (j <= i).astype(np.float32)
    mask = np.concatenate([tri, tri], axis=1).astype(bf)  # [128, 256]

    # v-tile statics: per (jb, pair) block of VBS cols:
    # col 0 = 1 (odd sumexp), 1:64 zeros, 64:192 v data (zero-init),
    # col 192 = 1 (even sumexp), col 193 pad
    blk = np.zeros(VBS, np.float32)
    blk[0] = blk[192] = 1.0
    vinit = np.tile(blk, (128, NTT * 2)).astype(bf)

    in_maps = []
    for b in range(B):
        xT = np.ascontiguousarray(x[b].T).astype(bf)  # [D, T]
        for g in range(4):
            hs = slice(g * HPG, (g + 1) * HPG)
            # v columns pair-swapped: [h1, h0, h3, h2]
            vord = [g * HPG + 1, g * HPG + 0, g * HPG + 3, g * HPG + 2]
            wqkv = np.concatenate(
                [Wq[:, hs].reshape(D, HPG * DH),
                 Wk[:, hs].reshape(D, HPG * DH),
                 Wv[:, vord].reshape(D, HPG * DH)], axis=1,
            ).astype(bf)
            wp = (Wproj[g * 256:(g + 1) * 256]
                  .reshape(2, 128, D).transpose(1, 0, 2).reshape(128, 2 * D)
                  ).astype(bf)
            in_maps.append({
                "xT": np.ascontiguousarray(xT),
                "wqkv": np.ascontiguousarray(wqkv),
                "wproj": np.ascontiguousarray(wp),
                "mask": mask,
                "vinit": vinit,
                "ones": np.ones((128, 128), np.float32),
            })
    return in_maps


def kernel(x, Wqkv, Wproj):
    from concourse.bass_utils import run_bass_kernel_spmd

    nc = _get_program()
    in_maps = _host_inputs(x, Wqkv, Wproj)
    res = run_bass_kernel_spmd(nc, in_maps, core_ids=list(range(8)))
    outs = [r["out"] for r in res.results]
    full = np.stack(
        [outs[b * 4] + outs[b * 4 + 1] + outs[b * 4 + 2] + outs[b * 4 + 3]
         for b in range(B)]
    ).astype(np.float32)
    return full
